# revision 11
# baseline (speedup 1.0000x reference)
"""BiAttention (binary attention transformer block) Trainium2 kernel.

Forward-pass reduction of the reference:
  - softmax cancels:  stop_gradient(binq - soft) + soft == binq  (forward)
  - sign() is invariant to the positive per-row qkv weight scale
So per batch element (one per NeuronCore, 8 cores data-parallel):
  bq,bk,bv = sign(x @ sign(Wqkv).T)   split into heads
  A        = (bq @ bk.T > 0)          in {0,1}
  oo       = A @ bv                   exact small integers
  out      = (oo @ sign(Wproj).T) * mean(|Wproj|,axis=1) + b_proj

Elementwise rebalance vs the earlier revision: the wsT8 (fp8 lo-pass weight)
scaled copies moved from the Activation engine to DVE (2x all-SBUF mode),
and half the projection bias adds moved to GPSIMD (f32 add), relieving the
Act-bound stretch between the qkv and attention phases.
"""

import numpy as np

import concourse.bacc as bacc
import concourse.bass as bass
import concourse.mybir as mybir
import concourse.tile as tile
from concourse.masks import make_identity

FP32 = mybir.dt.float32
FP16 = mybir.dt.float16
FP8 = mybir.dt.float8e4
FP8E5 = mybir.dt.float8e5
AF = mybir.ActivationFunctionType
ALU = mybir.AluOpType
DR = mybir.MatmulPerfMode.DoubleRow

B, N, C = 8, 1024, 768
H, D = 12, 64
C3 = 3 * C  # 2304
NK = C // 128  # 6 contraction chunks
NM = N // 128  # 8 token chunks
NOC = C3 // 128  # 18 qkv output chunks


QKV_MODE = "hilo"  # "hilo" (fp16 two-pass, exact) or "f32r" (single-pass float32r)
WT_MODE = "pe"  # "pe16" (sign on pool, fp16 transpose), "pe" (f32 transpose, sign on evac), "xbar"
SCORE_ORDER = "ncol"  # "ncol" (alternate PE row-groups: HW tile concurrency) or "h01" (serial)
LO_MODE = "fp8dr"  # qkv lo-pass: "fp16" (exact, 1.0 cyc/row) or "fp8dr" (e4m3 DoubleRow, 0.5)


def build_nc(repeat=1):
    nc = bacc.Bacc("TRN2", target_bir_lowering=False, debug=True)

    x_d = nc.dram_tensor("x", [N, C], FP32, kind="ExternalInput")
    wqkv_d = nc.dram_tensor("w_qkv", [C3, C], FP32, kind="ExternalInput")
    wproj_d = nc.dram_tensor("w_proj", [C, C], FP32, kind="ExternalInput")
    bproj_d = nc.dram_tensor("b_proj", [1, C], FP32, kind="ExternalInput")
    out_d = nc.dram_tensor("out", [N, C], FP32, kind="ExternalOutput")

    # DRAM views: row r = chunk*128 + partition
    x_v = x_d[:].rearrange("(c p) f -> p c f", p=128)  # [128, 8, 768]
    wqkv_v = wqkv_d[:].rearrange("(c p) f -> p c f", p=128)  # [128, 18, 768]
    wproj_v = wproj_d[:].rearrange("(c p) f -> p c f", p=128)  # [128, 6, 768]
    out_v = out_d[:].rearrange("(c p) f -> p c f", p=128)  # [128, 8, 768]

    with tile.TileContext(nc) as tc:
        for _rep in range(repeat):
            _emit_body(nc, tc, _rep, x_v, wqkv_v, wproj_v, bproj_d, out_v)

    nc.compile()
    return nc


def _emit_body(nc, tc, rep, x_v, wqkv_v, wproj_v, bproj_d, out_v):
    _p = f"r{rep}_"
    if True:
        with (
            tc.tile_pool(name=_p + "persist", bufs=1) as pp,
            tc.tile_pool(name=_p + "stage", bufs=3 if QKV_MODE == "f32r" else 5) as sp,
            tc.tile_pool(name=_p + "wstage", bufs=5 if QKV_MODE == "f32r" else 6) as wp,
            tc.tile_pool(name=_p + "qk", bufs=4 if QKV_MODE == "f32r" else 5) as qkp,
            tc.tile_pool(name=_p + "at", bufs=4) as atp,
            tc.tile_pool(name=_p + "outstage", bufs=2) as op,
            tc.tile_pool(name=_p + "w2pre", bufs=3 if WT_MODE == "pe16" else 6) as w2p,
            tc.tile_pool(name=_p + "wsg", bufs=3) as wsgp,
        ):
            # ---- persistent SBUF ----
            FPR = mybir.dt.float32r
            if QKV_MODE == "hilo":
                xT_hi = pp.tile([128, NK, N], FP16, tag="xT_hi")  # [c%128, c//128, n]
                wsT = pp.tile([128, NK, C3], FP16, tag="wsT")  # sign(wqkv).T
                if LO_MODE == "fp8dr":
                    # lo pass at fp8 DoubleRow: x_lo*2^9 (e4m3) x wsT*2^-9
                    # (e5m2) keeps products exactly x_lo*wsT
                    xT_lo8 = pp.tile([128, NK, N], FP8, tag="xT_lo8")
                    wsT8 = pp.tile([128, NK, C3], FP8E5, tag="wsT8")
                    qkv_srcs = (xT_hi,)
                else:
                    xT_lo = pp.tile([128, NK, N], FP16, tag="xT_lo")
                    qkv_srcs = (xT_hi, xT_lo)
            else:
                # single-pass f32r qkv: 1 cyc/row when the moving operand's
                # free dim >= 256 (vs 2 fp16 passes). walrus requires both
                # matmul operands 32-bit, so the sign-weights are f32r too.
                xT_r = pp.tile([128, NK, N], FPR, tag="xT_r")
                wsT = pp.tile([128, NK, C3], FPR, tag="wsT")
                qkv_srcs = (xT_r,)
            w2T = pp.tile([128, NK, C], FP16, tag="w2T")  # sign(wproj).T
            v_nat = pp.tile([128, NM, C], FP8, tag="v_nat")  # v, ±0.5, [m%128, m//128, hd]
            ooT = pp.tile([128, NK, N], FP16, tag="ooT")  # attn out transposed
            sc2_row = pp.tile([1, C], FP32, tag="sc2_row")  # mean|wproj| row
            sc2_rep = pp.tile([128, C], FP32, tag="sc2_rep")
            bias_row = pp.tile([1, C], FP32, tag="bias_row")
            bias_rep = pp.tile([128, C], FP32, tag="bias_rep")
            ident = pp.tile([128, 128], FP32, tag="ident")

            sigb = pp.tile([128, 1], FP32, tag="sigb")
            nc.gpsimd.memset(sigb[:], -32.0)
            make_identity(nc, ident[:])
            ident16 = pp.tile([128, 128], FP16, tag="ident16")
            nc.scalar.activation(ident16[:], ident[:], AF.Copy)
            nc.sync.dma_start(bias_row[:], bproj_d[:])

            # ---- prep phase: loads + PE transposes (PE is otherwise idle) ----
            misc_cm = [None]
            sc2_ps = None

            # x: load [n,c] chunks, transpose on PE, split into fp16 hi/lo
            # (own psum pool, closed after the loop to free banks)
            xtr_cm = tc.tile_pool(name=_p + "ps_xtr", bufs=2, space="PSUM")
            ps_xtr = xtr_cm.__enter__()
            for cc in range(NM):
                xs = sp.tile([128, C], FP32, tag="x_stage")
                nc.sync.dma_start(xs[:], x_v[:, cc, :])
                xtp = ps_xtr.tile([128, C], FP32, tag="tr_ps", name=f"xtr{cc}")
                for k in range(NK):
                    nc.tensor.transpose(
                        xtp[:, k * 128 : (k + 1) * 128],
                        xs[:, k * 128 : (k + 1) * 128],
                        ident[:],
                    )
                if QKV_MODE == "hilo" and LO_MODE == "fp8dr":
                    dst_hi = xT_hi[:, :, cc * 128 : (cc + 1) * 128]
                    nc.scalar.activation(dst_hi, xtp[:], AF.Copy)
                    xlo_t = sp.tile([128, C], FP16, tag="x_lo_t", name=f"xlo{cc}")
                    nc.vector.tensor_tensor(xlo_t[:], xtp[:], dst_hi, ALU.subtract)
                    nc.vector.tensor_scalar(
                        xT_lo8[:, :, cc * 128 : (cc + 1) * 128],
                        xlo_t[:],
                        512.0,
                        None,
                        ALU.mult,
                    )
                elif QKV_MODE == "hilo":
                    dst_hi = xT_hi[:, :, cc * 128 : (cc + 1) * 128]
                    dst_lo = xT_lo[:, :, cc * 128 : (cc + 1) * 128]
                    nc.scalar.activation(dst_hi, xtp[:], AF.Copy)
                    nc.vector.tensor_tensor(dst_lo, xtp[:], dst_hi, ALU.subtract)
                else:
                    nc.scalar.activation(
                        xT_r[:, :, cc * 128 : (cc + 1) * 128], xtp[:], AF.Copy
                    )
            xtr_cm.__exit__(None, None, None)

            tr_pool_cm = tc.tile_pool(name=_p + "ps_tr", bufs=3, space="PSUM")
            ps_tr = tr_pool_cm.__enter__()
            if WT_MODE == "pe16":
                misc_cm[0] = tc.tile_pool(name=_p + "ps_misc", bufs=1, space="PSUM")
                ps_misc = misc_cm[0].__enter__()
                sc2_ps = ps_misc.tile([1, C], FP32, tag="sc2_ps")

            # w_proj: sign+transpose; |.| row-means via accum  (emitted after
            # the w_qkv/v-part phase: its results are only needed by proj)
            def emit_w2_prep():
              nonlocal sc2_ps
              misc_cm[0] = tc.tile_pool(name=_p + "ps_misc", bufs=1, space="PSUM")
              ps_misc = misc_cm[0].__enter__()
              sc2_ps = ps_misc.tile([1, C], FP32, tag="sc2_ps")
              for cc in range(NK):
                w2s = w2s_tiles[cc]
                w2abs = sp.tile([128, C], FP16, tag="w2_abs", name=f"w2abs{cc}")
                sc2_col = sp.tile([128, 1], FP32, tag="sc2_col", name=f"sc2c{cc}")
                nc.scalar.activation(w2abs[:], w2s[:], AF.Abs, accum_out=sc2_col[:])
                nc.tensor.transpose(
                    sc2_ps[0:1, cc * 128 : (cc + 1) * 128], sc2_col[:], ident[:]
                )
                if WT_MODE == "pe":
                    w2tp = ps_tr.tile([128, C], FP32, tag="tr_ps", name=f"w2tr{cc}")
                    for k in range(NK):
                        nc.tensor.transpose(
                            w2tp[:, k * 128 : (k + 1) * 128],
                            w2s[:, k * 128 : (k + 1) * 128],
                            ident[:],
                        )
                    nc.scalar.activation(
                        w2T[:, :, cc * 128 : (cc + 1) * 128], w2tp[:], AF.Sign
                    )
                else:
                    w2sg = sp.tile([128, C], FP16, tag="w2_sign", name=f"w2sg{cc}")
                    nc.scalar.activation(w2sg[:], w2s[:], AF.Sign)
                    nc.sync.dma_start_transpose(
                        w2T[:, :, cc * 128 : (cc + 1) * 128], w2sg[:]
                    )
            def emit_w2_tail():
                # w2T is +-0.5 in pe16 mode: fold the 2x back into the scale
                f = (2.0 if WT_MODE == "pe16" else 1.0) / C
                nc.vector.tensor_scalar(sc2_row[:], sc2_ps[:], f, None, ALU.mult)
                nc.gpsimd.partition_broadcast(sc2_rep[:], sc2_row[:])
                nc.gpsimd.partition_broadcast(bias_rep[:], bias_row[:])

            # w_qkv: load, transpose on PE, sign -> fp16 wsT (v chunks first)
            oc_order = list(range(12, 18)) + [
                x for pair in zip(range(0, 6), range(6, 12)) for x in pair
            ]
            vpart_emitted = False
            vp_cm = None
            w2s_tiles = {}

            def load_w2(cc):
                t = w2p.tile([128, C], FP32, tag="w2_stage", name=f"w2s{cc}")
                nc.sync.dma_start(t[:], wproj_v[:, cc, :])
                if WT_MODE != "pe16":
                    w2s_tiles[cc] = t
                    return
                # inline: |.| row-sums, sign (+-0.5 on pool), fp16 transpose
                w2abs = sp.tile([128, C], FP16, tag="w2_abs", name=f"w2abs{cc}")
                sc2_col = sp.tile([128, 1], FP32, tag="sc2_col", name=f"sc2c{cc}")
                nc.scalar.activation(w2abs[:], t[:], AF.Abs, accum_out=sc2_col[:])
                nc.tensor.transpose(
                    sc2_ps[0:1, cc * 128 : (cc + 1) * 128], sc2_col[:], ident[:]
                )
                w2sg = wsgp.tile([128, C], FP16, tag="w_sign", name=f"w2sg{cc}")
                nc.gpsimd.tensor_scalar(w2sg[:], t[:], 0.0, 0.5, ALU.is_ge, ALU.subtract)
                w2tp = ps_tr.tile([128, C], FP16, tag="tr16_ps", name=f"w2tr{cc}")
                for k in range(NK):
                    nc.tensor.transpose(
                        w2tp[:, k * 128 : (k + 1) * 128],
                        w2sg[:, k * 128 : (k + 1) * 128],
                        ident16[:],
                    )
                nc.scalar.activation(
                    w2T[:, :, cc * 128 : (cc + 1) * 128], w2tp[:], AF.Copy
                )

            def emit_v_part():
                lo8 = QKV_MODE == "hilo" and LO_MODE == "fp8dr"
                for m in range(NM):
                    for half in range(2):
                        vp = ps_v.tile([128, 384], FP32, tag="v_ps", name=f"vps{m}_{half}")
                        ns = len(qkv_srcs)
                        for k in range(NK):
                            for si, src in enumerate(qkv_srcs):
                                nc.tensor.matmul(
                                    vp[:],
                                    lhsT=src[:, k, m * 128 : (m + 1) * 128],
                                    rhs=wsT[:, k, 1536 + half * 384 : 1536 + (half + 1) * 384],
                                    start=(k == 0 and si == 0),
                                    stop=(not lo8 and k == NK - 1 and si == ns - 1),
                                )
                        if lo8:
                            for j in range(NK // 2):
                                nc.tensor.matmul(
                                    vp[:],
                                    lhsT=xT_lo8[:, 2 * j : 2 * j + 2, m * 128 : (m + 1) * 128],
                                    rhs=wsT8[:, 2 * j : 2 * j + 2, 1536 + half * 384 : 1536 + (half + 1) * 384],
                                    perf_mode=DR,
                                    start=False,
                                    stop=(j == NK // 2 - 1),
                                )
                        nc.vector.tensor_scalar(
                            v_nat[:, m, half * 384 : (half + 1) * 384],
                            vp[:],
                            0.0,
                            0.5,
                            ALU.is_ge,
                            ALU.subtract,
                        )

            for i, oc in enumerate(oc_order):
                ws = wp.tile([128, C], FP32, tag="w_stage")
                nc.sync.dma_start(ws[:], wqkv_v[:, oc, :])
                if i % 3 == 2:
                    load_w2(i // 3)
                if WT_MODE == "pe16" and QKV_MODE != "f32r":
                    # sign (sbuf->sbuf, +-0.5), then fp16 PE transpose
                    # (1.0 cyc/row vs 2.0 for f32), evac copy on scalar.
                    # Early (v) chunks gate the v-part matmuls: sign them on
                    # the faster DVE; later chunks go to the idle pool engine.
                    wsg = wsgp.tile([128, C], FP16, tag="w_sign", name=f"wsg{oc}")
                    eng = nc.vector if i < 6 else nc.gpsimd
                    eng.tensor_scalar(
                        wsg[:], ws[:], 0.0, 0.5, ALU.is_ge, ALU.subtract
                    )
                    wtp = ps_tr.tile([128, C], FP16, tag="tr16_ps", name=f"wtr{oc}")
                    for k in range(NK):
                        nc.tensor.transpose(
                            wtp[:, k * 128 : (k + 1) * 128],
                            wsg[:, k * 128 : (k + 1) * 128],
                            ident16[:],
                        )
                    nc.scalar.activation(
                        wsT[:, :, oc * 128 : (oc + 1) * 128], wtp[:], AF.Copy
                    )
                elif WT_MODE == "pe" or QKV_MODE == "f32r":
                    wtp = ps_tr.tile([128, C], FP32, tag="tr_ps", name=f"wtr{oc}")
                    for k in range(NK):
                        nc.tensor.transpose(
                            wtp[:, k * 128 : (k + 1) * 128],
                            ws[:, k * 128 : (k + 1) * 128],
                            ident[:],
                        )
                    nc.scalar.activation(
                        wsT[:, :, oc * 128 : (oc + 1) * 128], wtp[:], AF.Sign
                    )
                else:
                    wsg = wp.tile([128, C], FP16, tag="w_sign")
                    nc.scalar.activation(wsg[:], ws[:], AF.Sign)
                    nc.sync.dma_start_transpose(
                        wsT[:, :, oc * 128 : (oc + 1) * 128], wsg[:]
                    )
                if QKV_MODE == "hilo" and LO_MODE == "fp8dr":
                    nc.vector.tensor_scalar(
                        wsT8[:, :, oc * 128 : (oc + 1) * 128],
                        wsT[:, :, oc * 128 : (oc + 1) * 128],
                        0.001953125,
                        None,
                        ALU.mult,
                    )
                if i == 5:
                    # v-slice of wsT complete: emit the v-part matmuls now so
                    # they overlap the remaining q/k chunk loads
                    vp_cm = tc.tile_pool(name=_p + "ps_v", bufs=2, space="PSUM")
                    ps_v = vp_cm.__enter__()
                    emit_v_part()
                    vpart_emitted = True
            assert vpart_emitted
            vp_cm.__exit__(None, None, None)
            if WT_MODE != "pe16":
                emit_w2_prep()
            emit_w2_tail()
            misc_cm[0].__exit__(None, None, None)
            tr_pool_cm.__exit__(None, None, None)

            # ---- per head-pair: q/k chunks, scores, binarize, A@V ----
            hp_psum_cms = [
                tc.tile_pool(name=_p + "ps_qk", bufs=2, space="PSUM"),
                tc.tile_pool(name=_p + "ps_s", bufs=2, space="PSUM"),
                tc.tile_pool(name=_p + "ps_oo", bufs=2, space="PSUM"),
            ]
            ps_qk, ps_s, ps_oo = [cm.__enter__() for cm in hp_psum_cms]
            bin_idx = 0
            qkTs = {}

            def qk_units(hp):
                """4 closures, one qk psum fill+Sign evac each; run inside
                the previous pair's scores m-loop to spread PE work across
                the binarize-paced stretch."""
                qkT = {}
                for role in ("q", "k"):
                    qkT[role] = qkp.tile(
                        [128, N], FP8, tag="qkT", name=f"qkT_{role}{hp}"
                    )
                qkTs[hp] = qkT
                lo8 = QKV_MODE == "hilo" and LO_MODE == "fp8dr"

                def make(role, oc, ncol):
                    def emit():
                        t = qkT[role]
                        qp = ps_qk.tile([128, 512], FP32, tag="qk_ps")
                        ns = len(qkv_srcs)
                        for k in range(NK):
                            for si, src in enumerate(qkv_srcs):
                                nc.tensor.matmul(
                                    qp[:],
                                    lhsT=wsT[:, k, oc * 128 : (oc + 1) * 128],
                                    rhs=src[:, k, ncol * 512 : (ncol + 1) * 512],
                                    start=(k == 0 and si == 0),
                                    stop=(not lo8 and k == NK - 1 and si == ns - 1),
                                )
                        if lo8:
                            for j in range(NK // 2):
                                nc.tensor.matmul(
                                    qp[:],
                                    lhsT=wsT8[:, 2 * j : 2 * j + 2, oc * 128 : (oc + 1) * 128],
                                    rhs=xT_lo8[:, 2 * j : 2 * j + 2, ncol * 512 : (ncol + 1) * 512],
                                    perf_mode=DR,
                                    start=False,
                                    stop=(j == NK // 2 - 1),
                                )
                        nc.scalar.activation(
                            t[:, ncol * 512 : (ncol + 1) * 512], qp[:], AF.Sign
                        )

                    return emit

                return [
                    make(role, oc, ncol)
                    for role, oc in (("q", hp), ("k", 6 + hp))
                    for ncol in range(2)
                ]

            def emit_qk(hp):
                for u in qk_units(hp):
                    u()

            emit_qk(0)

            def av_units(hp, at):
                """4 closures (h01, ncol): one A@V psum fill+evac each. Run
                interleaved inside the NEXT pair's scores m-loop so the PE
                has ready work while score psums wait on their binarize."""
                oo_tmp = op.tile([64, N], FP16, tag="oo_tmp", name=f"oo_tmp{hp}")

                def make(h01, ncol):
                    def emit():
                        h = 2 * hp + h01
                        oo_ps = ps_oo.tile(
                            [64, 512], FP32, tag="oo_ps", name=f"oo_ps{hp}_{h01}_{ncol}"
                        )
                        for j in range(4):
                            nc.tensor.matmul(
                                oo_ps[:],
                                lhsT=v_nat[:, 2 * j : 2 * j + 2, h * 64 : (h + 1) * 64],
                                rhs=at[h01][:, 2 * j : 2 * j + 2, ncol * 512 : (ncol + 1) * 512],
                                perf_mode=DR,
                                start=(j == 0),
                                stop=(j == 3),
                            )
                        # v was ±0.5 -> x2 recovers exact integer attention out;
                        # odd head's lanes land on partitions 64-127 via a
                        # small SBUF->SBUF partition-shift DMA
                        dsth = ooT[0:64, hp, :] if h01 == 0 else oo_tmp[:]
                        csl = dsth[:, ncol * 512 : (ncol + 1) * 512]
                        if ncol == 0:
                            nc.scalar.activation(csl, oo_ps[:], AF.Copy, scale=2.0)
                        else:
                            nc.vector.tensor_scalar(csl, oo_ps[:], 2.0, None, ALU.mult)
                        if h01 == 1 and ncol == 1:
                            nc.sync.dma_start(ooT[64:128, hp, :], oo_tmp[:])

                    return emit

                return [make(h01, ncol) for h01 in range(2) for ncol in range(2)]

            prev_av = []
            for hp in range(6):
                qkT = qkTs.pop(hp)
                next_qk = qk_units(hp + 1) if hp + 1 < 6 else []
                at = {}
                for h01 in range(2):
                    at[h01] = atp.tile([128, NM, N], FP8, tag="at", name=f"at{hp}_{h01}")
                for m in range(NM):
                    # ncol-major emission: adjacent matmuls target different PE
                    # row-groups (rows 0-63 vs 64-127), so the 64-deep reorder
                    # window can run them concurrently (2x on this phase).
                    sp_pss = [
                        ps_s.tile([128, N], FP32, tag="s_ps", name=f"sps{hp}_{m}_{h01}")
                        for h01 in range(2)
                    ]
                    mm_order = (
                        [(n, h) for n in range(2) for h in range(2)]
                        if SCORE_ORDER == "ncol"
                        else [(n, h) for h in range(2) for n in range(2)]
                    )
                    for ncol, h01 in mm_order:
                        ph = 64 * h01
                        nc.tensor.matmul(
                            sp_pss[h01][:, ncol * 512 : (ncol + 1) * 512],
                            lhsT=qkT["k"][ph : ph + 64, m * 128 : (m + 1) * 128],
                            rhs=qkT["q"][ph : ph + 64, ncol * 512 : (ncol + 1) * 512],
                            tile_position=(ph, 0),
                        )
                    for h01 in range(2):
                        dst = at[h01][:, m, :]
                        if bin_idx % 2 == 0:
                            nc.scalar.activation(
                                dst, sp_pss[h01][:], AF.Sigmoid, bias=sigb[:], scale=32.0
                            )
                        else:
                            nc.vector.tensor_scalar(dst, sp_pss[h01][:], 0.0, None, ALU.is_gt)
                        bin_idx += 1
                    if m % 2 == 0 and next_qk:
                        next_qk[m // 2]()
                    elif m % 2 == 1 and prev_av:
                        prev_av[m // 2]()

                prev_av = av_units(hp, at)
            for u in prev_av:
                u()
            for cm in reversed(hp_psum_cms):
                cm.__exit__(None, None, None)

        # ---- projection ----
        with (
            tc.tile_pool(name=_p + "proj_out", bufs=3) as pop,
            tc.tile_pool(name=_p + "ps_proj", bufs=2, space="PSUM") as ps_p,
        ):
            for m in range(NM):
                ot = pop.tile([128, C], FP32, tag="out_stage")
                for n0, nw in ((0, 512), (512, 256)):
                    pps = ps_p.tile([128, nw], FP32, tag=f"p_ps{n0}")
                    for k in range(NK):
                        nc.tensor.matmul(
                            pps[:],
                            lhsT=ooT[:, k, m * 128 : (m + 1) * 128],
                            rhs=w2T[:, k, n0 : n0 + nw],
                            start=(k == 0),
                            stop=(k == NK - 1),
                        )
                    nc.vector.scalar_tensor_tensor(
                        ot[:, n0 : n0 + nw],
                        pps[:],
                        1.0,
                        sc2_rep[:, n0 : n0 + nw],
                        ALU.bypass,
                        ALU.mult,
                    )
                eng = nc.gpsimd if (m % 2 == 0 and m < 6) else nc.vector
                eng.tensor_tensor(ot[:], ot[:], bias_rep[:], ALU.add)
                nc.sync.dma_start(out_v[:, m, :], ot[:])


_CACHE = {}


def _get_exec():
    """Build (once) and cache a jitted SPMD executable for the 8-core kernel."""
    if "exec" in _CACHE:
        return _CACHE["exec"]
    import jax
    import concourse.mybir as _mybir
    from jax.sharding import Mesh, PartitionSpec
    from jax.experimental.shard_map import shard_map
    from concourse.bass2jax import _bass_exec_p, install_neuronx_cc_hook

    nc = build_nc()
    install_neuronx_cc_hook()

    in_names, out_names, out_avals = [], [], []
    for alloc in nc.m.functions[0].allocations:
        if not isinstance(alloc, _mybir.MemoryLocationSet):
            continue
        name = alloc.memorylocations[0].name
        if alloc.kind == "ExternalInput":
            if name not in ("dbg_addr", "partition_id"):
                in_names.append(name)
        elif alloc.kind == "ExternalOutput":
            out_names.append(name)
            out_avals.append(
                jax.core.ShapedArray(tuple(alloc.tensor_shape), _mybir.dt.np(alloc.dtype))
            )
    if nc.dbg_addr is not None:
        in_names.append(nc.dbg_addr.name)
    n_params = len(in_names)
    n_outs = len(out_names)
    partition_name = nc.partition_id_tensor.name if nc.partition_id_tensor else None
    all_in_names = tuple(
        in_names + out_names + ([partition_name] if partition_name else [])
    )
    donate = tuple(range(n_params, n_params + n_outs))

    def _body(*args):
        operands = list(args)
        if partition_name is not None:
            from concourse.bass2jax import partition_id_tensor

            operands.append(partition_id_tensor())
        outs = _bass_exec_p.bind(
            *operands,
            out_avals=tuple(out_avals),
            in_names=all_in_names,
            out_names=tuple(out_names),
            lowering_input_output_aliases=(),
            sim_require_finite=True,
            sim_require_nnan=True,
            nc=nc,
        )
        return tuple(outs)

    devices = jax.devices()[:B]
    mesh = Mesh(np.array(devices), ("core",))
    in_specs = (PartitionSpec("core"),) * (n_params + n_outs)
    out_specs = (PartitionSpec("core"),) * n_outs
    sharded = jax.jit(
        shard_map(_body, mesh=mesh, in_specs=in_specs, out_specs=out_specs, check_rep=False),
        donate_argnums=donate,
        keep_unused=True,
    )
    _CACHE["exec"] = (sharded, in_names, out_names, out_avals, mesh)
    return _CACHE["exec"]


def _concat_inputs(x, w_qkv, w_proj, b_proj):
    """Per-core inputs concatenated along axis 0 (shard_map convention)."""
    x = np.asarray(x, np.float32)
    w_qkv = np.asarray(w_qkv, np.float32)
    w_proj = np.asarray(w_proj, np.float32)
    b_proj = np.asarray(b_proj, np.float32).reshape(1, C)
    per_core = {
        "x": [np.ascontiguousarray(x[b]) for b in range(B)],
        "w_qkv": [w_qkv] * B,
        "w_proj": [w_proj] * B,
        "b_proj": [b_proj] * B,
        "dbg_addr": [np.zeros((1, 2), np.uint32)] * B,
    }
    return per_core


def _zero_outs(out_names, out_avals):
    return [
        np.zeros((B * a.shape[0], *a.shape[1:]), a.dtype) for a in out_avals
    ]


def kernel(x, w_qkv, w_proj, b_proj):
    sharded, in_names, out_names, out_avals, mesh = _get_exec()
    per_core = _concat_inputs(x, w_qkv, w_proj, b_proj)
    concat_in = [np.concatenate(per_core[name], axis=0) for name in in_names]
    out_arrs = sharded(*concat_in, *_zero_outs(out_names, out_avals))
    i = out_names.index("out")
    a = out_avals[i]
    return np.asarray(out_arrs[i]).reshape(B, *a.shape)



# revision 12
# speedup vs baseline: 3.1390x; 3.1390x over previous
"""BiAttention (binary attention transformer block) Trainium2 kernel.

Forward-pass reduction of the reference:
  - softmax cancels:  stop_gradient(binq - soft) + soft == binq  (forward)
  - sign() is invariant to the positive per-row qkv weight scale
So per batch element (one per NeuronCore, 8 cores data-parallel):
  bq,bk,bv = sign(x @ sign(Wqkv).T)   split into heads
  A        = (bq @ bk.T > 0)          in {0,1}
  oo       = A @ bv                   exact small integers
  out      = (oo @ sign(Wproj).T) * mean(|Wproj|,axis=1) + b_proj

Elementwise rebalance vs the earlier revision: the wsT8 (fp8 lo-pass weight)
scaled copies moved from the Activation engine to DVE (2x all-SBUF mode),
and half the projection bias adds moved to GPSIMD (f32 add), relieving the
Act-bound stretch between the qkv and attention phases.
"""

import numpy as np

import concourse.bacc as bacc
import concourse.bass as bass
import concourse.mybir as mybir
import concourse.tile as tile
from concourse.masks import make_identity

FP32 = mybir.dt.float32
FP16 = mybir.dt.float16
FP8 = mybir.dt.float8e4
FP8E5 = mybir.dt.float8e5
AF = mybir.ActivationFunctionType
ALU = mybir.AluOpType
DR = mybir.MatmulPerfMode.DoubleRow

B, N, C = 8, 1024, 768
H, D = 12, 64
C3 = 3 * C  # 2304
NK = C // 128  # 6 contraction chunks
NM = N // 128  # 8 token chunks
NOC = C3 // 128  # 18 qkv output chunks


QKV_MODE = "hilo"  # "hilo" (fp16 two-pass, exact) or "f32r" (single-pass float32r)
WT_MODE = "pe"  # "pe16" (sign on pool, fp16 transpose), "pe" (f32 transpose, sign on evac), "xbar"
SCORE_ORDER = "ncol"  # "ncol" (alternate PE row-groups: HW tile concurrency) or "h01" (serial)
LO_MODE = "fp8dr"  # qkv lo-pass: "fp16" (exact, 1.0 cyc/row) or "fp8dr" (e4m3 DoubleRow, 0.5)


def build_nc(repeat=1):
    nc = bacc.Bacc("TRN2", target_bir_lowering=False, debug=True)

    x_d = nc.dram_tensor("x", [N, C], FP32, kind="ExternalInput")
    wqkv_d = nc.dram_tensor("w_qkv", [C3, C], FP32, kind="ExternalInput")
    wproj_d = nc.dram_tensor("w_proj", [C, C], FP32, kind="ExternalInput")
    bproj_d = nc.dram_tensor("b_proj", [1, C], FP32, kind="ExternalInput")
    out_d = nc.dram_tensor("out", [N, C], FP32, kind="ExternalOutput")

    # DRAM views: row r = chunk*128 + partition
    x_v = x_d[:].rearrange("(c p) f -> p c f", p=128)  # [128, 8, 768]
    wqkv_v = wqkv_d[:].rearrange("(c p) f -> p c f", p=128)  # [128, 18, 768]
    wproj_v = wproj_d[:].rearrange("(c p) f -> p c f", p=128)  # [128, 6, 768]
    out_v = out_d[:].rearrange("(c p) f -> p c f", p=128)  # [128, 8, 768]

    with tile.TileContext(nc) as tc:
        for _rep in range(repeat):
            _emit_body(nc, tc, _rep, x_v, wqkv_v, wproj_v, bproj_d, out_v)

    nc.compile()
    return nc


def _emit_body(nc, tc, rep, x_v, wqkv_v, wproj_v, bproj_d, out_v):
    _p = f"r{rep}_"
    if True:
        with (
            tc.tile_pool(name=_p + "persist", bufs=1) as pp,
            tc.tile_pool(name=_p + "stage", bufs=3 if QKV_MODE == "f32r" else 5) as sp,
            tc.tile_pool(name=_p + "wstage", bufs=5 if QKV_MODE == "f32r" else 6) as wp,
            tc.tile_pool(name=_p + "qk", bufs=4 if QKV_MODE == "f32r" else 5) as qkp,
            tc.tile_pool(name=_p + "at", bufs=4) as atp,
            tc.tile_pool(name=_p + "outstage", bufs=2) as op,
            tc.tile_pool(name=_p + "w2pre", bufs=3 if WT_MODE == "pe16" else 6) as w2p,
            tc.tile_pool(name=_p + "wsg", bufs=3) as wsgp,
        ):
            # ---- persistent SBUF ----
            FPR = mybir.dt.float32r
            if QKV_MODE == "hilo":
                xT_hi = pp.tile([128, NK, N], FP16, tag="xT_hi")  # [c%128, c//128, n]
                wsT = pp.tile([128, NK, C3], FP16, tag="wsT")  # sign(wqkv).T
                if LO_MODE == "fp8dr":
                    # lo pass at fp8 DoubleRow: x_lo*2^9 (e4m3) x wsT*2^-9
                    # (e5m2) keeps products exactly x_lo*wsT
                    xT_lo8 = pp.tile([128, NK, N], FP8, tag="xT_lo8")
                    wsT8 = pp.tile([128, NK, C3], FP8E5, tag="wsT8")
                    qkv_srcs = (xT_hi,)
                else:
                    xT_lo = pp.tile([128, NK, N], FP16, tag="xT_lo")
                    qkv_srcs = (xT_hi, xT_lo)
            else:
                # single-pass f32r qkv: 1 cyc/row when the moving operand's
                # free dim >= 256 (vs 2 fp16 passes). walrus requires both
                # matmul operands 32-bit, so the sign-weights are f32r too.
                xT_r = pp.tile([128, NK, N], FPR, tag="xT_r")
                wsT = pp.tile([128, NK, C3], FPR, tag="wsT")
                qkv_srcs = (xT_r,)
            w2T = pp.tile([128, NK, C], FP16, tag="w2T")  # sign(wproj).T
            v_nat = pp.tile([128, NM, C], FP8, tag="v_nat")  # v, ±0.5, [m%128, m//128, hd]
            ooT = pp.tile([128, NK, N], FP16, tag="ooT")  # attn out transposed
            sc2_row = pp.tile([1, C], FP32, tag="sc2_row")  # mean|wproj| row
            sc2_rep = pp.tile([128, C], FP32, tag="sc2_rep")
            bias_row = pp.tile([1, C], FP32, tag="bias_row")
            bias_rep = pp.tile([128, C], FP32, tag="bias_rep")
            ident = pp.tile([128, 128], FP32, tag="ident")

            sigb = pp.tile([128, 1], FP32, tag="sigb")
            nc.gpsimd.memset(sigb[:], -32.0)
            make_identity(nc, ident[:])
            ident16 = pp.tile([128, 128], FP16, tag="ident16")
            nc.scalar.activation(ident16[:], ident[:], AF.Copy)
            nc.sync.dma_start(bias_row[:], bproj_d[:])

            # ---- prep phase: loads + PE transposes (PE is otherwise idle) ----
            misc_cm = [None]
            sc2_ps = None

            # x: load [n,c] chunks, transpose on PE, split into fp16 hi/lo
            # (own psum pool, closed after the loop to free banks)
            xtr_cm = tc.tile_pool(name=_p + "ps_xtr", bufs=2, space="PSUM")
            ps_xtr = xtr_cm.__enter__()
            for cc in range(NM):
                xs = sp.tile([128, C], FP32, tag="x_stage")
                nc.sync.dma_start(xs[:], x_v[:, cc, :])
                xtp = ps_xtr.tile([128, C], FP32, tag="tr_ps", name=f"xtr{cc}")
                for k in range(NK):
                    nc.tensor.transpose(
                        xtp[:, k * 128 : (k + 1) * 128],
                        xs[:, k * 128 : (k + 1) * 128],
                        ident[:],
                    )
                if QKV_MODE == "hilo" and LO_MODE == "fp8dr":
                    dst_hi = xT_hi[:, :, cc * 128 : (cc + 1) * 128]
                    nc.scalar.activation(dst_hi, xtp[:], AF.Copy)
                    xlo_t = sp.tile([128, C], FP16, tag="x_lo_t", name=f"xlo{cc}")
                    nc.vector.tensor_tensor(xlo_t[:], xtp[:], dst_hi, ALU.subtract)
                    nc.vector.tensor_scalar(
                        xT_lo8[:, :, cc * 128 : (cc + 1) * 128],
                        xlo_t[:],
                        512.0,
                        None,
                        ALU.mult,
                    )
                elif QKV_MODE == "hilo":
                    dst_hi = xT_hi[:, :, cc * 128 : (cc + 1) * 128]
                    dst_lo = xT_lo[:, :, cc * 128 : (cc + 1) * 128]
                    nc.scalar.activation(dst_hi, xtp[:], AF.Copy)
                    nc.vector.tensor_tensor(dst_lo, xtp[:], dst_hi, ALU.subtract)
                else:
                    nc.scalar.activation(
                        xT_r[:, :, cc * 128 : (cc + 1) * 128], xtp[:], AF.Copy
                    )
            xtr_cm.__exit__(None, None, None)

            tr_pool_cm = tc.tile_pool(name=_p + "ps_tr", bufs=3, space="PSUM")
            ps_tr = tr_pool_cm.__enter__()
            if WT_MODE == "pe16":
                misc_cm[0] = tc.tile_pool(name=_p + "ps_misc", bufs=1, space="PSUM")
                ps_misc = misc_cm[0].__enter__()
                sc2_ps = ps_misc.tile([1, C], FP32, tag="sc2_ps")

            # w_proj: sign+transpose; |.| row-means via accum  (emitted after
            # the w_qkv/v-part phase: its results are only needed by proj)
            def emit_w2_prep():
              nonlocal sc2_ps
              misc_cm[0] = tc.tile_pool(name=_p + "ps_misc", bufs=1, space="PSUM")
              ps_misc = misc_cm[0].__enter__()
              sc2_ps = ps_misc.tile([1, C], FP32, tag="sc2_ps")
              for cc in range(NK):
                w2s = w2s_tiles[cc]
                w2abs = sp.tile([128, C], FP16, tag="w2_abs", name=f"w2abs{cc}")
                sc2_col = sp.tile([128, 1], FP32, tag="sc2_col", name=f"sc2c{cc}")
                nc.scalar.activation(w2abs[:], w2s[:], AF.Abs, accum_out=sc2_col[:])
                nc.tensor.transpose(
                    sc2_ps[0:1, cc * 128 : (cc + 1) * 128], sc2_col[:], ident[:]
                )
                if WT_MODE == "pe":
                    w2tp = ps_tr.tile([128, C], FP32, tag="tr_ps", name=f"w2tr{cc}")
                    for k in range(NK):
                        nc.tensor.transpose(
                            w2tp[:, k * 128 : (k + 1) * 128],
                            w2s[:, k * 128 : (k + 1) * 128],
                            ident[:],
                        )
                    nc.scalar.activation(
                        w2T[:, :, cc * 128 : (cc + 1) * 128], w2tp[:], AF.Sign
                    )
                else:
                    w2sg = sp.tile([128, C], FP16, tag="w2_sign", name=f"w2sg{cc}")
                    nc.scalar.activation(w2sg[:], w2s[:], AF.Sign)
                    nc.sync.dma_start_transpose(
                        w2T[:, :, cc * 128 : (cc + 1) * 128], w2sg[:]
                    )
            def emit_w2_tail():
                # w2T is +-0.5 in pe16 mode: fold the 2x back into the scale
                f = (2.0 if WT_MODE == "pe16" else 1.0) / C
                nc.vector.tensor_scalar(sc2_row[:], sc2_ps[:], f, None, ALU.mult)
                nc.gpsimd.partition_broadcast(sc2_rep[:], sc2_row[:])
                nc.gpsimd.partition_broadcast(bias_rep[:], bias_row[:])

            # w_qkv: load, transpose on PE, sign -> fp16 wsT (v chunks first)
            oc_order = list(range(12, 18)) + [
                x for pair in zip(range(0, 6), range(6, 12)) for x in pair
            ]
            vpart_emitted = False
            vp_cm = None
            w2s_tiles = {}

            def load_w2(cc):
                t = w2p.tile([128, C], FP32, tag="w2_stage", name=f"w2s{cc}")
                nc.sync.dma_start(t[:], wproj_v[:, cc, :])
                if WT_MODE != "pe16":
                    w2s_tiles[cc] = t
                    return
                # inline: |.| row-sums, sign (+-0.5 on pool), fp16 transpose
                w2abs = sp.tile([128, C], FP16, tag="w2_abs", name=f"w2abs{cc}")
                sc2_col = sp.tile([128, 1], FP32, tag="sc2_col", name=f"sc2c{cc}")
                nc.scalar.activation(w2abs[:], t[:], AF.Abs, accum_out=sc2_col[:])
                nc.tensor.transpose(
                    sc2_ps[0:1, cc * 128 : (cc + 1) * 128], sc2_col[:], ident[:]
                )
                w2sg = wsgp.tile([128, C], FP16, tag="w_sign", name=f"w2sg{cc}")
                nc.gpsimd.tensor_scalar(w2sg[:], t[:], 0.0, 0.5, ALU.is_ge, ALU.subtract)
                w2tp = ps_tr.tile([128, C], FP16, tag="tr16_ps", name=f"w2tr{cc}")
                for k in range(NK):
                    nc.tensor.transpose(
                        w2tp[:, k * 128 : (k + 1) * 128],
                        w2sg[:, k * 128 : (k + 1) * 128],
                        ident16[:],
                    )
                nc.scalar.activation(
                    w2T[:, :, cc * 128 : (cc + 1) * 128], w2tp[:], AF.Copy
                )

            def emit_v_part():
                lo8 = QKV_MODE == "hilo" and LO_MODE == "fp8dr"
                for m in range(NM):
                    for half in range(2):
                        vp = ps_v.tile([128, 384], FP32, tag="v_ps", name=f"vps{m}_{half}")
                        ns = len(qkv_srcs)
                        for k in range(NK):
                            for si, src in enumerate(qkv_srcs):
                                nc.tensor.matmul(
                                    vp[:],
                                    lhsT=src[:, k, m * 128 : (m + 1) * 128],
                                    rhs=wsT[:, k, 1536 + half * 384 : 1536 + (half + 1) * 384],
                                    start=(k == 0 and si == 0),
                                    stop=(not lo8 and k == NK - 1 and si == ns - 1),
                                )
                        if lo8:
                            for j in range(NK // 2):
                                nc.tensor.matmul(
                                    vp[:],
                                    lhsT=xT_lo8[:, 2 * j : 2 * j + 2, m * 128 : (m + 1) * 128],
                                    rhs=wsT8[:, 2 * j : 2 * j + 2, 1536 + half * 384 : 1536 + (half + 1) * 384],
                                    perf_mode=DR,
                                    start=False,
                                    stop=(j == NK // 2 - 1),
                                )
                        nc.vector.tensor_scalar(
                            v_nat[:, m, half * 384 : (half + 1) * 384],
                            vp[:],
                            0.0,
                            0.5,
                            ALU.is_ge,
                            ALU.subtract,
                        )

            for i, oc in enumerate(oc_order):
                ws = wp.tile([128, C], FP32, tag="w_stage")
                nc.sync.dma_start(ws[:], wqkv_v[:, oc, :])
                if i % 3 == 2:
                    load_w2(i // 3)
                if WT_MODE == "pe16" and QKV_MODE != "f32r":
                    # sign (sbuf->sbuf, +-0.5), then fp16 PE transpose
                    # (1.0 cyc/row vs 2.0 for f32), evac copy on scalar.
                    # Early (v) chunks gate the v-part matmuls: sign them on
                    # the faster DVE; later chunks go to the idle pool engine.
                    wsg = wsgp.tile([128, C], FP16, tag="w_sign", name=f"wsg{oc}")
                    eng = nc.vector if i < 6 else nc.gpsimd
                    eng.tensor_scalar(
                        wsg[:], ws[:], 0.0, 0.5, ALU.is_ge, ALU.subtract
                    )
                    wtp = ps_tr.tile([128, C], FP16, tag="tr16_ps", name=f"wtr{oc}")
                    for k in range(NK):
                        nc.tensor.transpose(
                            wtp[:, k * 128 : (k + 1) * 128],
                            wsg[:, k * 128 : (k + 1) * 128],
                            ident16[:],
                        )
                    nc.scalar.activation(
                        wsT[:, :, oc * 128 : (oc + 1) * 128], wtp[:], AF.Copy
                    )
                elif WT_MODE == "pe" or QKV_MODE == "f32r":
                    wtp = ps_tr.tile([128, C], FP32, tag="tr_ps", name=f"wtr{oc}")
                    for k in range(NK):
                        nc.tensor.transpose(
                            wtp[:, k * 128 : (k + 1) * 128],
                            ws[:, k * 128 : (k + 1) * 128],
                            ident[:],
                        )
                    nc.scalar.activation(
                        wsT[:, :, oc * 128 : (oc + 1) * 128], wtp[:], AF.Sign
                    )
                else:
                    wsg = wp.tile([128, C], FP16, tag="w_sign")
                    nc.scalar.activation(wsg[:], ws[:], AF.Sign)
                    nc.sync.dma_start_transpose(
                        wsT[:, :, oc * 128 : (oc + 1) * 128], wsg[:]
                    )
                if QKV_MODE == "hilo" and LO_MODE == "fp8dr":
                    nc.vector.tensor_scalar(
                        wsT8[:, :, oc * 128 : (oc + 1) * 128],
                        wsT[:, :, oc * 128 : (oc + 1) * 128],
                        0.001953125,
                        None,
                        ALU.mult,
                    )
                if i == 5:
                    # v-slice of wsT complete: emit the v-part matmuls now so
                    # they overlap the remaining q/k chunk loads
                    vp_cm = tc.tile_pool(name=_p + "ps_v", bufs=2, space="PSUM")
                    ps_v = vp_cm.__enter__()
                    emit_v_part()
                    vpart_emitted = True
            assert vpart_emitted
            vp_cm.__exit__(None, None, None)
            if WT_MODE != "pe16":
                emit_w2_prep()
            emit_w2_tail()
            misc_cm[0].__exit__(None, None, None)
            tr_pool_cm.__exit__(None, None, None)

            # ---- per head-pair: q/k chunks, scores, binarize, A@V ----
            hp_psum_cms = [
                tc.tile_pool(name=_p + "ps_qk", bufs=2, space="PSUM"),
                tc.tile_pool(name=_p + "ps_s", bufs=2, space="PSUM"),
                tc.tile_pool(name=_p + "ps_oo", bufs=2, space="PSUM"),
            ]
            ps_qk, ps_s, ps_oo = [cm.__enter__() for cm in hp_psum_cms]
            bin_idx = 0
            qkTs = {}

            def emit_qk(hp):
                qkT = {}
                lo8 = QKV_MODE == "hilo" and LO_MODE == "fp8dr"
                for role, oc in (("q", hp), ("k", 6 + hp)):
                    t = qkp.tile([128, N], FP8, tag="qkT", name=f"qkT_{role}{hp}")
                    qkT[role] = t
                    for ncol in range(2):
                        qp = ps_qk.tile([128, 512], FP32, tag="qk_ps")
                        ns = len(qkv_srcs)
                        for k in range(NK):
                            for si, src in enumerate(qkv_srcs):
                                nc.tensor.matmul(
                                    qp[:],
                                    lhsT=wsT[:, k, oc * 128 : (oc + 1) * 128],
                                    rhs=src[:, k, ncol * 512 : (ncol + 1) * 512],
                                    start=(k == 0 and si == 0),
                                    stop=(not lo8 and k == NK - 1 and si == ns - 1),
                                )
                        if lo8:
                            for j in range(NK // 2):
                                nc.tensor.matmul(
                                    qp[:],
                                    lhsT=wsT8[:, 2 * j : 2 * j + 2, oc * 128 : (oc + 1) * 128],
                                    rhs=xT_lo8[:, 2 * j : 2 * j + 2, ncol * 512 : (ncol + 1) * 512],
                                    perf_mode=DR,
                                    start=False,
                                    stop=(j == NK // 2 - 1),
                                )
                        nc.scalar.activation(
                            t[:, ncol * 512 : (ncol + 1) * 512], qp[:], AF.Sign
                        )
                qkTs[hp] = qkT

            emit_qk(0)

            def av_units(hp, at):
                """4 closures (h01, ncol): one A@V psum fill+evac each. Run
                interleaved inside the NEXT pair's scores m-loop so the PE
                has ready work while score psums wait on their binarize."""
                oo_tmp = op.tile([64, N], FP16, tag="oo_tmp", name=f"oo_tmp{hp}")

                def make(h01, ncol):
                    def emit():
                        h = 2 * hp + h01
                        oo_ps = ps_oo.tile(
                            [64, 512], FP32, tag="oo_ps", name=f"oo_ps{hp}_{h01}_{ncol}"
                        )
                        for j in range(4):
                            nc.tensor.matmul(
                                oo_ps[:],
                                lhsT=v_nat[:, 2 * j : 2 * j + 2, h * 64 : (h + 1) * 64],
                                rhs=at[h01][:, 2 * j : 2 * j + 2, ncol * 512 : (ncol + 1) * 512],
                                perf_mode=DR,
                                start=(j == 0),
                                stop=(j == 3),
                            )
                        # v was ±0.5 -> x2 recovers exact integer attention out;
                        # odd head's lanes land on partitions 64-127 via a
                        # small SBUF->SBUF partition-shift DMA
                        dsth = ooT[0:64, hp, :] if h01 == 0 else oo_tmp[:]
                        csl = dsth[:, ncol * 512 : (ncol + 1) * 512]
                        if ncol == 0:
                            nc.scalar.activation(csl, oo_ps[:], AF.Copy, scale=2.0)
                        else:
                            nc.vector.tensor_scalar(csl, oo_ps[:], 2.0, None, ALU.mult)
                        if h01 == 1 and ncol == 1:
                            nc.sync.dma_start(ooT[64:128, hp, :], oo_tmp[:])

                    return emit

                return [make(h01, ncol) for h01 in range(2) for ncol in range(2)]

            prev_av = []
            for hp in range(6):
                qkT = qkTs.pop(hp)
                at = {}
                for h01 in range(2):
                    at[h01] = atp.tile([128, NM, N], FP8, tag="at", name=f"at{hp}_{h01}")
                for m in range(NM):
                    # ncol-major emission: adjacent matmuls target different PE
                    # row-groups (rows 0-63 vs 64-127), so the 64-deep reorder
                    # window can run them concurrently (2x on this phase).
                    sp_pss = [
                        ps_s.tile([128, N], FP32, tag="s_ps", name=f"sps{hp}_{m}_{h01}")
                        for h01 in range(2)
                    ]
                    mm_order = (
                        [(n, h) for n in range(2) for h in range(2)]
                        if SCORE_ORDER == "ncol"
                        else [(n, h) for h in range(2) for n in range(2)]
                    )
                    for ncol, h01 in mm_order:
                        ph = 64 * h01
                        nc.tensor.matmul(
                            sp_pss[h01][:, ncol * 512 : (ncol + 1) * 512],
                            lhsT=qkT["k"][ph : ph + 64, m * 128 : (m + 1) * 128],
                            rhs=qkT["q"][ph : ph + 64, ncol * 512 : (ncol + 1) * 512],
                            tile_position=(ph, 0),
                        )
                    for h01 in range(2):
                        dst = at[h01][:, m, :]
                        if bin_idx % 2 == 0:
                            nc.scalar.activation(
                                dst, sp_pss[h01][:], AF.Sigmoid, bias=sigb[:], scale=32.0
                            )
                        else:
                            nc.vector.tensor_scalar(dst, sp_pss[h01][:], 0.0, None, ALU.is_gt)
                        bin_idx += 1
                    if m % 2 == 1 and prev_av:
                        prev_av[m // 2]()

                if hp + 1 < 6:
                    emit_qk(hp + 1)
                prev_av = av_units(hp, at)
            for u in prev_av:
                u()
            for cm in reversed(hp_psum_cms):
                cm.__exit__(None, None, None)

        # ---- projection ----
        with (
            tc.tile_pool(name=_p + "proj_out", bufs=3) as pop,
            tc.tile_pool(name=_p + "ps_proj", bufs=2, space="PSUM") as ps_p,
        ):
            for m in range(NM):
                ot = pop.tile([128, C], FP32, tag="out_stage")
                for n0, nw in ((0, 512), (512, 256)):
                    pps = ps_p.tile([128, nw], FP32, tag=f"p_ps{n0}")
                    for k in range(NK):
                        nc.tensor.matmul(
                            pps[:],
                            lhsT=ooT[:, k, m * 128 : (m + 1) * 128],
                            rhs=w2T[:, k, n0 : n0 + nw],
                            start=(k == 0),
                            stop=(k == NK - 1),
                        )
                    nc.vector.scalar_tensor_tensor(
                        ot[:, n0 : n0 + nw],
                        pps[:],
                        1.0,
                        sc2_rep[:, n0 : n0 + nw],
                        ALU.bypass,
                        ALU.mult,
                    )
                eng = nc.gpsimd if (m % 2 == 0 and m < 6) else nc.vector
                eng.tensor_tensor(ot[:], ot[:], bias_rep[:], ALU.add)
                nc.sync.dma_start(out_v[:, m, :], ot[:])


_CACHE = {}


def _get_exec():
    """Build (once) and cache a jitted SPMD executable for the 8-core kernel."""
    if "exec" in _CACHE:
        return _CACHE["exec"]
    import jax
    import concourse.mybir as _mybir
    from jax.sharding import Mesh, PartitionSpec
    from jax.experimental.shard_map import shard_map
    from concourse.bass2jax import _bass_exec_p, install_neuronx_cc_hook

    nc = build_nc()
    install_neuronx_cc_hook()

    in_names, out_names, out_avals = [], [], []
    for alloc in nc.m.functions[0].allocations:
        if not isinstance(alloc, _mybir.MemoryLocationSet):
            continue
        name = alloc.memorylocations[0].name
        if alloc.kind == "ExternalInput":
            if name not in ("dbg_addr", "partition_id"):
                in_names.append(name)
        elif alloc.kind == "ExternalOutput":
            out_names.append(name)
            out_avals.append(
                jax.core.ShapedArray(tuple(alloc.tensor_shape), _mybir.dt.np(alloc.dtype))
            )
    if nc.dbg_addr is not None:
        in_names.append(nc.dbg_addr.name)
    n_params = len(in_names)
    n_outs = len(out_names)
    partition_name = nc.partition_id_tensor.name if nc.partition_id_tensor else None
    all_in_names = tuple(
        in_names + out_names + ([partition_name] if partition_name else [])
    )
    donate = tuple(range(n_params, n_params + n_outs))

    def _body(*args):
        operands = list(args)
        if partition_name is not None:
            from concourse.bass2jax import partition_id_tensor

            operands.append(partition_id_tensor())
        outs = _bass_exec_p.bind(
            *operands,
            out_avals=tuple(out_avals),
            in_names=all_in_names,
            out_names=tuple(out_names),
            lowering_input_output_aliases=(),
            sim_require_finite=True,
            sim_require_nnan=True,
            nc=nc,
        )
        return tuple(outs)

    devices = jax.devices()[:B]
    mesh = Mesh(np.array(devices), ("core",))
    in_specs = (PartitionSpec("core"),) * (n_params + n_outs)
    out_specs = (PartitionSpec("core"),) * n_outs
    sharded = jax.jit(
        shard_map(_body, mesh=mesh, in_specs=in_specs, out_specs=out_specs, check_rep=False),
        donate_argnums=donate,
        keep_unused=True,
    )
    _CACHE["exec"] = (sharded, in_names, out_names, out_avals, mesh)
    return _CACHE["exec"]


def _concat_inputs(x, w_qkv, w_proj, b_proj):
    """Per-core inputs concatenated along axis 0 (shard_map convention)."""
    x = np.asarray(x, np.float32)
    w_qkv = np.asarray(w_qkv, np.float32)
    w_proj = np.asarray(w_proj, np.float32)
    b_proj = np.asarray(b_proj, np.float32).reshape(1, C)
    per_core = {
        "x": [np.ascontiguousarray(x[b]) for b in range(B)],
        "w_qkv": [w_qkv] * B,
        "w_proj": [w_proj] * B,
        "b_proj": [b_proj] * B,
        "dbg_addr": [np.zeros((1, 2), np.uint32)] * B,
    }
    return per_core


def _zero_outs(out_names, out_avals):
    return [
        np.zeros((B * a.shape[0], *a.shape[1:]), a.dtype) for a in out_avals
    ]


def kernel(x, w_qkv, w_proj, b_proj):
    sharded, in_names, out_names, out_avals, mesh = _get_exec()
    per_core = _concat_inputs(x, w_qkv, w_proj, b_proj)
    concat_in = [np.concatenate(per_core[name], axis=0) for name in in_names]
    out_arrs = sharded(*concat_in, *_zero_outs(out_names, out_avals))
    i = out_names.index("out")
    a = out_avals[i]
    return np.asarray(out_arrs[i]).reshape(B, *a.shape)



# revision 15
# speedup vs baseline: 3.6449x; 1.1612x over previous
"""BiAttention (binary attention transformer block) Trainium2 kernel.

Forward-pass reduction of the reference:
  - softmax cancels:  stop_gradient(binq - soft) + soft == binq  (forward)
  - sign() is invariant to the positive per-row qkv weight scale
So per batch element (one per NeuronCore, 8 cores data-parallel):
  bq,bk,bv = sign(x @ sign(Wqkv).T)   split into heads
  A        = (bq @ bk.T > 0)          in {0,1}
  oo       = A @ bv                   exact small integers
  out      = (oo @ sign(Wproj).T) * mean(|Wproj|,axis=1) + b_proj

Elementwise rebalance vs the earlier revision: the wsT8 (fp8 lo-pass weight)
scaled copies moved from the Activation engine to DVE (2x all-SBUF mode),
and half the projection bias adds moved to GPSIMD (f32 add), relieving the
Act-bound stretch between the qkv and attention phases.
"""

import numpy as np

import concourse.bacc as bacc
import concourse.bass as bass
import concourse.mybir as mybir
import concourse.tile as tile
from concourse.masks import make_identity

FP32 = mybir.dt.float32
FP16 = mybir.dt.float16
FP8 = mybir.dt.float8e4
FP8E5 = mybir.dt.float8e5
AF = mybir.ActivationFunctionType
ALU = mybir.AluOpType
DR = mybir.MatmulPerfMode.DoubleRow

B, N, C = 8, 1024, 768
H, D = 12, 64
C3 = 3 * C  # 2304
NK = C // 128  # 6 contraction chunks
NM = N // 128  # 8 token chunks
NOC = C3 // 128  # 18 qkv output chunks


QKV_MODE = "hilo"  # "hilo" (fp16 two-pass, exact) or "f32r" (single-pass float32r)
WT_MODE = "pe"  # "pe16" (sign on pool, fp16 transpose), "pe" (f32 transpose, sign on evac), "xbar"
SCORE_ORDER = "ncol"  # "ncol" (alternate PE row-groups: HW tile concurrency) or "h01" (serial)
LO_MODE = "fp8dr"  # qkv lo-pass: "fp16" (exact, 1.0 cyc/row) or "fp8dr" (e4m3 DoubleRow, 0.5)


def build_nc(repeat=1):
    nc = bacc.Bacc("TRN2", target_bir_lowering=False, debug=True)

    x_d = nc.dram_tensor("x", [N, C], FP32, kind="ExternalInput")
    wqkv_d = nc.dram_tensor("w_qkv", [C3, C], FP32, kind="ExternalInput")
    wproj_d = nc.dram_tensor("w_proj", [C, C], FP32, kind="ExternalInput")
    bproj_d = nc.dram_tensor("b_proj", [1, C], FP32, kind="ExternalInput")
    out_d = nc.dram_tensor("out", [N, C], FP32, kind="ExternalOutput")

    # DRAM views: row r = chunk*128 + partition
    x_v = x_d[:].rearrange("(c p) f -> p c f", p=128)  # [128, 8, 768]
    wqkv_v = wqkv_d[:].rearrange("(c p) f -> p c f", p=128)  # [128, 18, 768]
    wproj_v = wproj_d[:].rearrange("(c p) f -> p c f", p=128)  # [128, 6, 768]
    out_v = out_d[:].rearrange("(c p) f -> p c f", p=128)  # [128, 8, 768]

    with tile.TileContext(nc) as tc:
        for _rep in range(repeat):
            _emit_body(nc, tc, _rep, x_v, wqkv_v, wproj_v, bproj_d, out_v)

    nc.compile()
    return nc


def _emit_body(nc, tc, rep, x_v, wqkv_v, wproj_v, bproj_d, out_v):
    _p = f"r{rep}_"
    if True:
        with (
            tc.tile_pool(name=_p + "persist", bufs=1) as pp,
            tc.tile_pool(name=_p + "stage", bufs=3 if QKV_MODE == "f32r" else 5) as sp,
            tc.tile_pool(name=_p + "wstage", bufs=5 if QKV_MODE == "f32r" else 6) as wp,
            tc.tile_pool(name=_p + "qk", bufs=4 if QKV_MODE == "f32r" else 5) as qkp,
            tc.tile_pool(name=_p + "at", bufs=4) as atp,
            tc.tile_pool(name=_p + "outstage", bufs=2) as op,
            tc.tile_pool(name=_p + "w2pre", bufs=3 if WT_MODE == "pe16" else 6) as w2p,
            tc.tile_pool(name=_p + "wsg", bufs=3) as wsgp,
        ):
            # ---- persistent SBUF ----
            FPR = mybir.dt.float32r
            if QKV_MODE == "hilo":
                xT_hi = pp.tile([128, NK, N], FP16, tag="xT_hi")  # [c%128, c//128, n]
                wsT = pp.tile([128, NK, C3], FP16, tag="wsT")  # sign(wqkv).T
                if LO_MODE == "fp8dr":
                    # lo pass at fp8 DoubleRow: x_lo*2^9 (e4m3) x wsT*2^-9
                    # (e5m2) keeps products exactly x_lo*wsT
                    xT_lo8 = pp.tile([128, NK, N], FP8, tag="xT_lo8")
                    wsT8 = pp.tile([128, NK, C3], FP8E5, tag="wsT8")
                    qkv_srcs = (xT_hi,)
                else:
                    xT_lo = pp.tile([128, NK, N], FP16, tag="xT_lo")
                    qkv_srcs = (xT_hi, xT_lo)
            else:
                # single-pass f32r qkv: 1 cyc/row when the moving operand's
                # free dim >= 256 (vs 2 fp16 passes). walrus requires both
                # matmul operands 32-bit, so the sign-weights are f32r too.
                xT_r = pp.tile([128, NK, N], FPR, tag="xT_r")
                wsT = pp.tile([128, NK, C3], FPR, tag="wsT")
                qkv_srcs = (xT_r,)
            w2T = pp.tile([128, NK, C], FP16, tag="w2T")  # sign(wproj).T
            v_nat = pp.tile([128, NM, C], FP8, tag="v_nat")  # v, ±0.5, [m%128, m//128, hd]
            ooT = pp.tile([128, NK, N], FP16, tag="ooT")  # attn out transposed
            sc2_row = pp.tile([1, C], FP32, tag="sc2_row")  # mean|wproj| row
            sc2_rep = pp.tile([128, C], FP32, tag="sc2_rep")
            bias_row = pp.tile([1, C], FP32, tag="bias_row")
            bias_rep = pp.tile([128, C], FP32, tag="bias_rep")
            ident = pp.tile([128, 128], FP32, tag="ident")

            sigb = pp.tile([128, 1], FP32, tag="sigb")
            nc.gpsimd.memset(sigb[:], -32.0)
            make_identity(nc, ident[:])
            ident16 = pp.tile([128, 128], FP16, tag="ident16")
            nc.scalar.activation(ident16[:], ident[:], AF.Copy)
            nc.sync.dma_start(bias_row[:], bproj_d[:])

            # ---- prep phase: loads + PE transposes (PE is otherwise idle) ----
            misc_cm = [None]
            sc2_ps = None

            # x: load [n,c] chunks, transpose on PE, split into fp16 hi/lo.
            # ps_tr opens first (reduced to 2 bufs) so the first weight
            # blocks can be prepped interleaved with the x chunks: the PE
            # would otherwise idle ~10us waiting for weight DMAs after the
            # x transposes finish.
            tr_pool_cm = tc.tile_pool(name=_p + "ps_tr", bufs=2, space="PSUM")
            ps_tr = tr_pool_cm.__enter__()
            xtr_cm = tc.tile_pool(name=_p + "ps_xtr", bufs=2, space="PSUM")
            ps_xtr = xtr_cm.__enter__()

            def x_chunk(cc):
                xs = sp.tile([128, C], FP32, tag="x_stage")
                nc.sync.dma_start(xs[:], x_v[:, cc, :])
                xtp = ps_xtr.tile([128, C], FP32, tag="tr_ps", name=f"xtr{cc}")
                for k in range(NK):
                    nc.tensor.transpose(
                        xtp[:, k * 128 : (k + 1) * 128],
                        xs[:, k * 128 : (k + 1) * 128],
                        ident[:],
                    )
                if QKV_MODE == "hilo" and LO_MODE == "fp8dr":
                    dst_hi = xT_hi[:, :, cc * 128 : (cc + 1) * 128]
                    nc.scalar.activation(dst_hi, xtp[:], AF.Copy)
                    xlo_t = sp.tile([128, C], FP16, tag="x_lo_t", name=f"xlo{cc}")
                    nc.vector.tensor_tensor(xlo_t[:], xtp[:], dst_hi, ALU.subtract)
                    nc.vector.tensor_scalar(
                        xT_lo8[:, :, cc * 128 : (cc + 1) * 128],
                        xlo_t[:],
                        512.0,
                        None,
                        ALU.mult,
                    )
                elif QKV_MODE == "hilo":
                    dst_hi = xT_hi[:, :, cc * 128 : (cc + 1) * 128]
                    dst_lo = xT_lo[:, :, cc * 128 : (cc + 1) * 128]
                    nc.scalar.activation(dst_hi, xtp[:], AF.Copy)
                    nc.vector.tensor_tensor(dst_lo, xtp[:], dst_hi, ALU.subtract)
                else:
                    nc.scalar.activation(
                        xT_r[:, :, cc * 128 : (cc + 1) * 128], xtp[:], AF.Copy
                    )

            if WT_MODE == "pe16":
                misc_cm[0] = tc.tile_pool(name=_p + "ps_misc", bufs=1, space="PSUM")
                ps_misc = misc_cm[0].__enter__()
                sc2_ps = ps_misc.tile([1, C], FP32, tag="sc2_ps")

            # w_proj: sign+transpose; |.| row-means via accum  (emitted after
            # the w_qkv/v-part phase: its results are only needed by proj)
            def emit_w2_prep():
              nonlocal sc2_ps
              misc_cm[0] = tc.tile_pool(name=_p + "ps_misc", bufs=1, space="PSUM")
              ps_misc = misc_cm[0].__enter__()
              sc2_ps = ps_misc.tile([1, C], FP32, tag="sc2_ps")
              for cc in range(NK):
                w2s = w2s_tiles[cc]
                w2abs = sp.tile([128, C], FP16, tag="w2_abs", name=f"w2abs{cc}")
                sc2_col = sp.tile([128, 1], FP32, tag="sc2_col", name=f"sc2c{cc}")
                nc.scalar.activation(w2abs[:], w2s[:], AF.Abs, accum_out=sc2_col[:])
                nc.tensor.transpose(
                    sc2_ps[0:1, cc * 128 : (cc + 1) * 128], sc2_col[:], ident[:]
                )
                if WT_MODE == "pe":
                    w2tp = ps_tr.tile([128, C], FP32, tag="tr_ps", name=f"w2tr{cc}")
                    for k in range(NK):
                        nc.tensor.transpose(
                            w2tp[:, k * 128 : (k + 1) * 128],
                            w2s[:, k * 128 : (k + 1) * 128],
                            ident[:],
                        )
                    nc.scalar.activation(
                        w2T[:, :, cc * 128 : (cc + 1) * 128], w2tp[:], AF.Sign
                    )
                else:
                    w2sg = sp.tile([128, C], FP16, tag="w2_sign", name=f"w2sg{cc}")
                    nc.scalar.activation(w2sg[:], w2s[:], AF.Sign)
                    nc.sync.dma_start_transpose(
                        w2T[:, :, cc * 128 : (cc + 1) * 128], w2sg[:]
                    )
            def emit_w2_tail():
                # w2T is +-0.5 in pe16 mode: fold the 2x back into the scale
                f = (2.0 if WT_MODE == "pe16" else 1.0) / C
                nc.vector.tensor_scalar(sc2_row[:], sc2_ps[:], f, None, ALU.mult)
                nc.gpsimd.partition_broadcast(sc2_rep[:], sc2_row[:])
                nc.gpsimd.partition_broadcast(bias_rep[:], bias_row[:])

            # w_qkv: load, transpose on PE, sign -> fp16 wsT (v chunks first)
            oc_order = list(range(12, 18)) + [
                x for pair in zip(range(0, 6), range(6, 12)) for x in pair
            ]
            vpart_emitted = False
            vp_cm = None
            ps_v = None
            w2s_tiles = {}

            def load_w2(cc):
                t = w2p.tile([128, C], FP32, tag="w2_stage", name=f"w2s{cc}")
                nc.sync.dma_start(t[:], wproj_v[:, cc, :])
                if WT_MODE != "pe16":
                    w2s_tiles[cc] = t
                    return
                # inline: |.| row-sums, sign (+-0.5 on pool), fp16 transpose
                w2abs = sp.tile([128, C], FP16, tag="w2_abs", name=f"w2abs{cc}")
                sc2_col = sp.tile([128, 1], FP32, tag="sc2_col", name=f"sc2c{cc}")
                nc.scalar.activation(w2abs[:], t[:], AF.Abs, accum_out=sc2_col[:])
                nc.tensor.transpose(
                    sc2_ps[0:1, cc * 128 : (cc + 1) * 128], sc2_col[:], ident[:]
                )
                w2sg = wsgp.tile([128, C], FP16, tag="w_sign", name=f"w2sg{cc}")
                nc.gpsimd.tensor_scalar(w2sg[:], t[:], 0.0, 0.5, ALU.is_ge, ALU.subtract)
                w2tp = ps_tr.tile([128, C], FP16, tag="tr16_ps", name=f"w2tr{cc}")
                for k in range(NK):
                    nc.tensor.transpose(
                        w2tp[:, k * 128 : (k + 1) * 128],
                        w2sg[:, k * 128 : (k + 1) * 128],
                        ident16[:],
                    )
                nc.scalar.activation(
                    w2T[:, :, cc * 128 : (cc + 1) * 128], w2tp[:], AF.Copy
                )

            def emit_v_part():
                lo8 = QKV_MODE == "hilo" and LO_MODE == "fp8dr"
                for m in range(NM):
                    for half in range(2):
                        vp = ps_v.tile([128, 384], FP32, tag="v_ps", name=f"vps{m}_{half}")
                        ns = len(qkv_srcs)
                        for k in range(NK):
                            for si, src in enumerate(qkv_srcs):
                                nc.tensor.matmul(
                                    vp[:],
                                    lhsT=src[:, k, m * 128 : (m + 1) * 128],
                                    rhs=wsT[:, k, 1536 + half * 384 : 1536 + (half + 1) * 384],
                                    start=(k == 0 and si == 0),
                                    stop=(not lo8 and k == NK - 1 and si == ns - 1),
                                )
                        if lo8:
                            for j in range(NK // 2):
                                nc.tensor.matmul(
                                    vp[:],
                                    lhsT=xT_lo8[:, 2 * j : 2 * j + 2, m * 128 : (m + 1) * 128],
                                    rhs=wsT8[:, 2 * j : 2 * j + 2, 1536 + half * 384 : 1536 + (half + 1) * 384],
                                    perf_mode=DR,
                                    start=False,
                                    stop=(j == NK // 2 - 1),
                                )
                        nc.vector.tensor_scalar(
                            v_nat[:, m, half * 384 : (half + 1) * 384],
                            vp[:],
                            0.0,
                            0.5,
                            ALU.is_ge,
                            ALU.subtract,
                        )

            def w_iter(i, oc):
                nonlocal vp_cm, ps_v, vpart_emitted
                ws = wp.tile([128, C], FP32, tag="w_stage")
                nc.sync.dma_start(ws[:], wqkv_v[:, oc, :])
                if i % 3 == 2:
                    load_w2(i // 3)
                if WT_MODE == "pe16" and QKV_MODE != "f32r":
                    # sign (sbuf->sbuf, +-0.5), then fp16 PE transpose
                    # (1.0 cyc/row vs 2.0 for f32), evac copy on scalar.
                    # Early (v) chunks gate the v-part matmuls: sign them on
                    # the faster DVE; later chunks go to the idle pool engine.
                    wsg = wsgp.tile([128, C], FP16, tag="w_sign", name=f"wsg{oc}")
                    eng = nc.vector if i < 6 else nc.gpsimd
                    eng.tensor_scalar(
                        wsg[:], ws[:], 0.0, 0.5, ALU.is_ge, ALU.subtract
                    )
                    wtp = ps_tr.tile([128, C], FP16, tag="tr16_ps", name=f"wtr{oc}")
                    for k in range(NK):
                        nc.tensor.transpose(
                            wtp[:, k * 128 : (k + 1) * 128],
                            wsg[:, k * 128 : (k + 1) * 128],
                            ident16[:],
                        )
                    nc.scalar.activation(
                        wsT[:, :, oc * 128 : (oc + 1) * 128], wtp[:], AF.Copy
                    )
                elif WT_MODE == "pe" or QKV_MODE == "f32r":
                    wtp = ps_tr.tile([128, C], FP32, tag="tr_ps", name=f"wtr{oc}")
                    for k in range(NK):
                        nc.tensor.transpose(
                            wtp[:, k * 128 : (k + 1) * 128],
                            ws[:, k * 128 : (k + 1) * 128],
                            ident[:],
                        )
                    nc.scalar.activation(
                        wsT[:, :, oc * 128 : (oc + 1) * 128], wtp[:], AF.Sign
                    )
                else:
                    wsg = wp.tile([128, C], FP16, tag="w_sign")
                    nc.scalar.activation(wsg[:], ws[:], AF.Sign)
                    nc.sync.dma_start_transpose(
                        wsT[:, :, oc * 128 : (oc + 1) * 128], wsg[:]
                    )
                if QKV_MODE == "hilo" and LO_MODE == "fp8dr":
                    nc.vector.tensor_scalar(
                        wsT8[:, :, oc * 128 : (oc + 1) * 128],
                        wsT[:, :, oc * 128 : (oc + 1) * 128],
                        0.001953125,
                        None,
                        ALU.mult,
                    )
                if i == 5:
                    # v-slice of wsT complete: emit the v-part matmuls now so
                    # they overlap the remaining q/k chunk loads
                    vp_cm = tc.tile_pool(name=_p + "ps_v", bufs=2, space="PSUM")
                    ps_v = vp_cm.__enter__()
                    emit_v_part()
                    vpart_emitted = True
            # interleave: first x chunks, then the first three weight
            # blocks spread among the remaining x chunks (DMA + PE transposes
            # overlap), then the rest of the weight pipeline.
            for cc in range(3):
                x_chunk(cc)
            for j in range(3):
                w_iter(j, oc_order[j])
                x_chunk(3 + j)
            x_chunk(6)
            x_chunk(7)
            xtr_cm.__exit__(None, None, None)
            for i in range(3, len(oc_order)):
                w_iter(i, oc_order[i])
            assert vpart_emitted
            vp_cm.__exit__(None, None, None)
            if WT_MODE != "pe16":
                emit_w2_prep()
            emit_w2_tail()
            misc_cm[0].__exit__(None, None, None)
            tr_pool_cm.__exit__(None, None, None)

            # ---- per head-pair: q/k chunks, scores, binarize, A@V ----
            hp_psum_cms = [
                tc.tile_pool(name=_p + "ps_qk", bufs=2, space="PSUM"),
                tc.tile_pool(name=_p + "ps_s", bufs=2, space="PSUM"),
                tc.tile_pool(name=_p + "ps_oo", bufs=2, space="PSUM"),
            ]
            ps_qk, ps_s, ps_oo = [cm.__enter__() for cm in hp_psum_cms]
            bin_idx = 0
            qkTs = {}

            def emit_qk(hp):
                qkT = {}
                lo8 = QKV_MODE == "hilo" and LO_MODE == "fp8dr"
                for role, oc in (("q", hp), ("k", 6 + hp)):
                    t = qkp.tile([128, N], FP8, tag="qkT", name=f"qkT_{role}{hp}")
                    qkT[role] = t
                    for ncol in range(2):
                        qp = ps_qk.tile([128, 512], FP32, tag="qk_ps")
                        ns = len(qkv_srcs)
                        for k in range(NK):
                            for si, src in enumerate(qkv_srcs):
                                nc.tensor.matmul(
                                    qp[:],
                                    lhsT=wsT[:, k, oc * 128 : (oc + 1) * 128],
                                    rhs=src[:, k, ncol * 512 : (ncol + 1) * 512],
                                    start=(k == 0 and si == 0),
                                    stop=(not lo8 and k == NK - 1 and si == ns - 1),
                                )
                        if lo8:
                            for j in range(NK // 2):
                                nc.tensor.matmul(
                                    qp[:],
                                    lhsT=wsT8[:, 2 * j : 2 * j + 2, oc * 128 : (oc + 1) * 128],
                                    rhs=xT_lo8[:, 2 * j : 2 * j + 2, ncol * 512 : (ncol + 1) * 512],
                                    perf_mode=DR,
                                    start=False,
                                    stop=(j == NK // 2 - 1),
                                )
                        nc.scalar.activation(
                            t[:, ncol * 512 : (ncol + 1) * 512], qp[:], AF.Sign
                        )
                qkTs[hp] = qkT

            emit_qk(0)

            def av_units(hp, at):
                """4 closures (h01, ncol): one A@V psum fill+evac each. Run
                interleaved inside the NEXT pair's scores m-loop so the PE
                has ready work while score psums wait on their binarize."""
                oo_tmp = op.tile([64, N], FP16, tag="oo_tmp", name=f"oo_tmp{hp}")

                def make(h01, ncol):
                    def emit():
                        h = 2 * hp + h01
                        oo_ps = ps_oo.tile(
                            [64, 512], FP32, tag="oo_ps", name=f"oo_ps{hp}_{h01}_{ncol}"
                        )
                        for j in range(4):
                            nc.tensor.matmul(
                                oo_ps[:],
                                lhsT=v_nat[:, 2 * j : 2 * j + 2, h * 64 : (h + 1) * 64],
                                rhs=at[h01][:, 2 * j : 2 * j + 2, ncol * 512 : (ncol + 1) * 512],
                                perf_mode=DR,
                                start=(j == 0),
                                stop=(j == 3),
                            )
                        # v was ±0.5 -> x2 recovers exact integer attention out;
                        # odd head's lanes land on partitions 64-127 via a
                        # small SBUF->SBUF partition-shift DMA
                        dsth = ooT[0:64, hp, :] if h01 == 0 else oo_tmp[:]
                        csl = dsth[:, ncol * 512 : (ncol + 1) * 512]
                        if ncol == 0:
                            nc.scalar.activation(csl, oo_ps[:], AF.Copy, scale=2.0)
                        else:
                            nc.vector.tensor_scalar(csl, oo_ps[:], 2.0, None, ALU.mult)
                        if h01 == 1 and ncol == 1:
                            nc.sync.dma_start(ooT[64:128, hp, :], oo_tmp[:])

                    return emit

                return [make(h01, ncol) for h01 in range(2) for ncol in range(2)]

            prev_av = []
            for hp in range(6):
                qkT = qkTs.pop(hp)
                at = {}
                for h01 in range(2):
                    at[h01] = atp.tile([128, NM, N], FP8, tag="at", name=f"at{hp}_{h01}")
                for m in range(NM):
                    # ncol-major emission: adjacent matmuls target different PE
                    # row-groups (rows 0-63 vs 64-127), so the 64-deep reorder
                    # window can run them concurrently (2x on this phase).
                    sp_pss = [
                        ps_s.tile([128, N], FP32, tag="s_ps", name=f"sps{hp}_{m}_{h01}")
                        for h01 in range(2)
                    ]
                    mm_order = (
                        [(n, h) for n in range(2) for h in range(2)]
                        if SCORE_ORDER == "ncol"
                        else [(n, h) for h in range(2) for n in range(2)]
                    )
                    for ncol, h01 in mm_order:
                        ph = 64 * h01
                        nc.tensor.matmul(
                            sp_pss[h01][:, ncol * 512 : (ncol + 1) * 512],
                            lhsT=qkT["k"][ph : ph + 64, m * 128 : (m + 1) * 128],
                            rhs=qkT["q"][ph : ph + 64, ncol * 512 : (ncol + 1) * 512],
                            tile_position=(ph, 0),
                        )
                    for h01 in range(2):
                        dst = at[h01][:, m, :]
                        if bin_idx % 2 == 0:
                            nc.scalar.activation(
                                dst, sp_pss[h01][:], AF.Sigmoid, bias=sigb[:], scale=32.0
                            )
                        else:
                            nc.vector.tensor_scalar(dst, sp_pss[h01][:], 0.0, None, ALU.is_gt)
                        bin_idx += 1
                    if m % 2 == 1 and prev_av:
                        prev_av[m // 2]()

                if hp + 1 < 6:
                    emit_qk(hp + 1)
                prev_av = av_units(hp, at)
            for u in prev_av:
                u()
            for cm in reversed(hp_psum_cms):
                cm.__exit__(None, None, None)

        # ---- projection ----
        with (
            tc.tile_pool(name=_p + "proj_out", bufs=3) as pop,
            tc.tile_pool(name=_p + "ps_proj", bufs=2, space="PSUM") as ps_p,
        ):
            for m in range(NM):
                ot = pop.tile([128, C], FP32, tag="out_stage")
                for n0, nw in ((0, 512), (512, 256)):
                    pps = ps_p.tile([128, nw], FP32, tag=f"p_ps{n0}")
                    for k in range(NK):
                        nc.tensor.matmul(
                            pps[:],
                            lhsT=ooT[:, k, m * 128 : (m + 1) * 128],
                            rhs=w2T[:, k, n0 : n0 + nw],
                            start=(k == 0),
                            stop=(k == NK - 1),
                        )
                    nc.vector.scalar_tensor_tensor(
                        ot[:, n0 : n0 + nw],
                        pps[:],
                        1.0,
                        sc2_rep[:, n0 : n0 + nw],
                        ALU.bypass,
                        ALU.mult,
                    )
                eng = nc.gpsimd if (m % 2 == 0 and m < 6) else nc.vector
                eng.tensor_tensor(ot[:], ot[:], bias_rep[:], ALU.add)
                nc.sync.dma_start(out_v[:, m, :], ot[:])


_CACHE = {}


def _get_exec():
    """Build (once) and cache a jitted SPMD executable for the 8-core kernel."""
    if "exec" in _CACHE:
        return _CACHE["exec"]
    import jax
    import concourse.mybir as _mybir
    from jax.sharding import Mesh, PartitionSpec
    from jax.experimental.shard_map import shard_map
    from concourse.bass2jax import _bass_exec_p, install_neuronx_cc_hook

    nc = build_nc()
    install_neuronx_cc_hook()

    in_names, out_names, out_avals = [], [], []
    for alloc in nc.m.functions[0].allocations:
        if not isinstance(alloc, _mybir.MemoryLocationSet):
            continue
        name = alloc.memorylocations[0].name
        if alloc.kind == "ExternalInput":
            if name not in ("dbg_addr", "partition_id"):
                in_names.append(name)
        elif alloc.kind == "ExternalOutput":
            out_names.append(name)
            out_avals.append(
                jax.core.ShapedArray(tuple(alloc.tensor_shape), _mybir.dt.np(alloc.dtype))
            )
    if nc.dbg_addr is not None:
        in_names.append(nc.dbg_addr.name)
    n_params = len(in_names)
    n_outs = len(out_names)
    partition_name = nc.partition_id_tensor.name if nc.partition_id_tensor else None
    all_in_names = tuple(
        in_names + out_names + ([partition_name] if partition_name else [])
    )
    donate = tuple(range(n_params, n_params + n_outs))

    def _body(*args):
        operands = list(args)
        if partition_name is not None:
            from concourse.bass2jax import partition_id_tensor

            operands.append(partition_id_tensor())
        outs = _bass_exec_p.bind(
            *operands,
            out_avals=tuple(out_avals),
            in_names=all_in_names,
            out_names=tuple(out_names),
            lowering_input_output_aliases=(),
            sim_require_finite=True,
            sim_require_nnan=True,
            nc=nc,
        )
        return tuple(outs)

    devices = jax.devices()[:B]
    mesh = Mesh(np.array(devices), ("core",))
    in_specs = (PartitionSpec("core"),) * (n_params + n_outs)
    out_specs = (PartitionSpec("core"),) * n_outs
    sharded = jax.jit(
        shard_map(_body, mesh=mesh, in_specs=in_specs, out_specs=out_specs, check_rep=False),
        donate_argnums=donate,
        keep_unused=True,
    )
    _CACHE["exec"] = (sharded, in_names, out_names, out_avals, mesh)
    return _CACHE["exec"]


def _concat_inputs(x, w_qkv, w_proj, b_proj):
    """Per-core inputs concatenated along axis 0 (shard_map convention)."""
    x = np.asarray(x, np.float32)
    w_qkv = np.asarray(w_qkv, np.float32)
    w_proj = np.asarray(w_proj, np.float32)
    b_proj = np.asarray(b_proj, np.float32).reshape(1, C)
    per_core = {
        "x": [np.ascontiguousarray(x[b]) for b in range(B)],
        "w_qkv": [w_qkv] * B,
        "w_proj": [w_proj] * B,
        "b_proj": [b_proj] * B,
        "dbg_addr": [np.zeros((1, 2), np.uint32)] * B,
    }
    return per_core


def _zero_outs(out_names, out_avals):
    return [
        np.zeros((B * a.shape[0], *a.shape[1:]), a.dtype) for a in out_avals
    ]


def kernel(x, w_qkv, w_proj, b_proj):
    sharded, in_names, out_names, out_avals, mesh = _get_exec()
    per_core = _concat_inputs(x, w_qkv, w_proj, b_proj)
    concat_in = [np.concatenate(per_core[name], axis=0) for name in in_names]
    out_arrs = sharded(*concat_in, *_zero_outs(out_names, out_avals))
    i = out_names.index("out")
    a = out_avals[i]
    return np.asarray(out_arrs[i]).reshape(B, *a.shape)

